# revision 28
# baseline (speedup 1.0000x reference)
"""Trainium2 Bass kernel for GAT-style GNN message passing (edge softmax).

Contract: kernel(**inputs) takes FULL unsharded numpy inputs, distributes
across 8 NeuronCores internally, returns FULL output.

Sharding: edges sorted by dst and partitioned by dst range (6250 nodes per
core) -> every per-destination segment reduction is core-local. Node
features/weights replicated.

v2 design notes (vs v1 baseline):
- All matmuls use f16/bf16/f8 operands (fp32 matmul is 4 cyc/row on PE).
- edge_feat projections (e_bias/gates) computed in the main block loop
  (no separate stage + DRAM round trip).
- f32->f16 conversion happens inside the DMA (SWDGE cast) - no on-chip
  cast instructions.
- q and v are consumed directly from PSUM by the DVE ops that need them
  (the op IS the drain); k is drained to f16 SBUF split ACT/gpsimd.
- Edge-major one-hot (agg lhsT) shipped from host as f8 instead of an
  on-device is_equal build.
- k for own nodes kept resident in SBUF (no DRAM round trip).
- LayerNorm restructured: Square(bias=-mu, accum) + folded affine.
"""

import os
import sys

sys.path.insert(0, "/opt/trn_rl_repo")

import numpy as np

import concourse.bass as bass
import concourse.mybir as mybir
import concourse.tile as tile
from concourse import bacc
from concourse import bass_utils
from concourse.masks import make_identity

F32 = mybir.dt.float32
F16 = mybir.dt.float16
BF16 = mybir.dt.bfloat16
F8 = mybir.dt.float8e4
I32 = mybir.dt.int32
AF = mybir.ActivationFunctionType
OP = mybir.AluOpType

D = 128
H = 8
HD = 16
EPS = 1e-5

N_NODES = 50000
N_EDGES = 800000
CORES = 8
NPC = N_NODES // CORES      # nodes per core = 6250
BLK = 125                   # dst nodes per block
NBLK = NPC // BLK           # 50 blocks per core

# If "1": host stages the big per-edge tables (edge_feat, feat[src]) in
# fp16 (same values the device math would see after its own cast).
HOST_F16 = os.environ.get("KERNEL_HOST_F16", "1") == "1"


def _force_act_set():
    """Pin every ACTIVATE to the natural_log_exp_and_others table so the
    kernel pays one ACT_TABLE_LOAD instead of hundreds."""
    from concourse import hw_specs

    if getattr(bacc, "_act_set_forced", False):
        return
    real = hw_specs.get_activation_tables

    def patched(arch):
        t = dict(real(arch))
        keep = "natural_log_exp_and_others"
        return {name: (fns if name == keep else set()) for name, fns in t.items()}

    bacc.get_activation_tables = patched
    bacc._act_set_forced = True


def build_program(cfg):
    _force_act_set()
    cores = cfg["cores"]
    npc = cfg["npc"]
    nblk = cfg["nblk"]
    blk = cfg["blk"]
    M = cfg["M"]
    CAP = M * 128
    EPC = nblk * CAP
    host_f16 = cfg["host_f16"]
    big_dt = F16 if host_f16 else F32

    nc = bacc.Bacc(
        "TRN2", target_bir_lowering=False, debug=False, num_devices=cores
    )

    # ---- I/O ----
    eft_d = nc.dram_tensor("eft", [D, EPC], big_dt, kind="ExternalInput").ap()
    fslotT_d = nc.dram_tensor("fslotT", [D, EPC], big_dt, kind="ExternalInput").ap()
    ohT_d = nc.dram_tensor("ohT", [nblk, 128, M, 128], F8, kind="ExternalInput").ap()
    ohE_d = nc.dram_tensor("ohE", [nblk, 128, M, 128], F8, kind="ExternalInput").ap()
    featT_d = nc.dram_tensor("featT", [D, npc], big_dt, kind="ExternalInput").ap()
    w_in = {}
    for name in ("Wq", "Wk", "Wv", "Wo", "Wskip", "W1", "W2", "WoT", "WskipT"):
        w_in[name] = nc.dram_tensor(name, [D, D], F32, kind="ExternalInput").ap()
    w_in["We"] = nc.dram_tensor("We", [D, H], F32, kind="ExternalInput").ap()
    w_in["Wg"] = nc.dram_tensor("Wg", [D, H], F32, kind="ExternalInput").ap()
    w_in["Wgate"] = nc.dram_tensor("Wgate", [3 * D, 1], F32, kind="ExternalInput").ap()
    for name in ("ln1_g", "ln1_b", "ln2_g", "ln2_b"):
        w_in[name] = nc.dram_tensor(name, [D], F32, kind="ExternalInput").ap()
    out_d = nc.dram_tensor("out", [npc, D], F32, kind="ExternalOutput").ap()

    with tile.TileContext(nc) as tc:
        import contextlib

        ctx = contextlib.ExitStack()
        with ctx:
            consts = ctx.enter_context(tc.tile_pool(name="consts", bufs=1))

            # ---------- setup ----------
            ident16 = consts.tile([128, 128], F16)
            make_identity(nc, ident16[:])

            ones_row = consts.tile([1, 128], F32)
            nc.vector.memset(ones_row[:], 1.0)

            zrow = consts.tile([128, D], F32)
            nc.vector.memset(zrow[:], 0.0)

            const2 = consts.tile([128, 2], F32)
            nc.vector.memset(const2[:, 0:1], 0.0)
            nc.vector.memset(const2[:, 1:2], EPS)
            nc.const_aps.aps[(F32, 0.0)] = const2[:, 0:1]
            nc.const_aps.aps[(F32, EPS)] = const2[:, 1:2]

            lnrow = consts.tile([1, 4 * D], F32)
            for i, name in enumerate(("ln1_g", "ln1_b", "ln2_g", "ln2_b")):
                nc.sync.dma_start(
                    out=lnrow[:, i * D : (i + 1) * D], in_=w_in[name][None, :]
                )
            lnb = consts.tile([128, 4 * D], F32)

            wqv32 = consts.tile([D, 2 * D], F32)
            nc.sync.dma_start(out=wqv32[:, 0:D], in_=w_in["Wq"][:])
            nc.sync.dma_start(out=wqv32[:, D : 2 * D], in_=w_in["Wv"][:])
            wqv16 = consts.tile([D, 2 * D], F16)
            nc.vector.tensor_copy(out=wqv16[:], in_=wqv32[:])
            wk16 = consts.tile([D, D], F16)
            nc.gpsimd.dma_start(out=wk16[:], in_=w_in["Wk"][:])
            weg32 = consts.tile([D, 2 * H], F32)
            nc.sync.dma_start(out=weg32[:, 0:H], in_=w_in["We"][:])
            nc.sync.dma_start(out=weg32[:, H : 2 * H], in_=w_in["Wg"][:])
            weg16 = consts.tile([D, 2 * H], F16)
            nc.vector.tensor_copy(out=weg16[:], in_=weg32[:])
            w1_16 = consts.tile([D, D], F16)
            nc.gpsimd.dma_start(out=w1_16[:], in_=w_in["W1"][:])
            w2_16 = consts.tile([D, D], F16)
            nc.gpsimd.dma_start(out=w2_16[:], in_=w_in["W2"][:])

            # gate vector folding: gate_pre = agg@(Wo@A) + feat@(Wskip@B)
            wg3 = consts.tile([128, 3], F32)
            nc.sync.dma_start(
                out=wg3[:], in_=w_in["Wgate"].rearrange("(t p) c -> p (t c)", p=128)
            )
            ab = consts.tile([128, 2], F32)
            nc.vector.tensor_add(out=ab[:, 0:1], in0=wg3[:, 0:1], in1=wg3[:, 2:3])
            nc.vector.tensor_sub(out=ab[:, 1:2], in0=wg3[:, 1:2], in1=wg3[:, 2:3])

            wot_s = consts.tile([D, D], F32)
            nc.sync.dma_start(out=wot_s[:], in_=w_in["WoT"][:])
            wskipt_s = consts.tile([D, D], F32)
            nc.sync.dma_start(out=wskipt_s[:], in_=w_in["WskipT"][:])

            rhs_o32 = consts.tile([D, D + 1], F32)
            nc.sync.dma_start(out=rhs_o32[:, 0:D], in_=w_in["Wo"][:])
            rhs_s32 = consts.tile([D, D + 1], F32)
            nc.sync.dma_start(out=rhs_s32[:, 1 : D + 1], in_=w_in["Wskip"][:])

            with tc.tile_pool(name="psum_setup", bufs=1, space="PSUM") as pss:
                ps_ln = pss.tile([128, 4 * D], F32, tag="ln")
                nc.tensor.matmul(
                    out=ps_ln[:], lhsT=ones_row[:], rhs=lnrow[:], start=True, stop=True
                )
                nc.vector.tensor_copy(out=lnb[:], in_=ps_ln[:])

                ps_c = pss.tile([128, 2], F32, tag="c")
                nc.tensor.matmul(
                    out=ps_c[:, 0:1], lhsT=wot_s[:], rhs=ab[:, 0:1],
                    start=True, stop=True,
                )
                nc.tensor.matmul(
                    out=ps_c[:, 1:2], lhsT=wskipt_s[:], rhs=ab[:, 1:2],
                    start=True, stop=True,
                )
                nc.vector.tensor_copy(out=rhs_o32[:, D : D + 1], in_=ps_c[:, 0:1])
                nc.vector.tensor_copy(out=rhs_s32[:, 0:1], in_=ps_c[:, 1:2])

            rhs_o16 = consts.tile([D, D + 1], F16)
            nc.vector.tensor_copy(out=rhs_o16[:], in_=rhs_o32[:])
            rhs_s16 = consts.tile([D, D + 1], F16)
            nc.vector.tensor_copy(out=rhs_s16[:], in_=rhs_s32[:])

            featT16 = consts.tile([D, npc], F16)
            if host_f16:
                nc.sync.dma_start(out=featT16[:], in_=featT_d[:])
            else:
                nc.gpsimd.dma_start(out=featT16[:], in_=featT_d[:])

            # ---------- stage 1: k for own nodes, SBUF-resident ----------
            # k_all[n_local, b, :] = (feat @ Wk)[b*blk + n_local, :]
            k_all = consts.tile([blk, nblk, D], F16)
            with tc.tile_pool(name="k1_ps", bufs=2, space="PSUM") as kps:
                for g0 in range(0, nblk, 4):
                    ng = min(4, nblk - g0)
                    ps_k1 = kps.tile([blk, 4, D], F32, tag="k1")
                    for j in range(ng):
                        b = g0 + j
                        nc.tensor.matmul(
                            out=ps_k1[:, j, :],
                            lhsT=featT16[:, b * blk : (b + 1) * blk],
                            rhs=wk16[:],
                            start=True, stop=True,
                        )
                    nc.scalar.activation(
                        out=k_all[:, g0 : g0 + ng, :], in_=ps_k1[:, 0:ng, :],
                        func=AF.Copy,
                    )

            # ---------- main loop ----------
            sb2 = ctx.enter_context(tc.tile_pool(name="p2_sb", bufs=3))
            sb2a = ctx.enter_context(tc.tile_pool(name="p2_sba", bufs=2))
            epi = ctx.enter_context(tc.tile_pool(name="epi_sb", bufs=2))
            # PSUM bank packing (8 banks x 2KB): q/v share one rotating tag
            # (2 banks), k 2 banks, eg+agg packed in one tile (2 banks),
            # tr 1 bank, rs 1 bank.
            ps_qvp = ctx.enter_context(tc.tile_pool(name="ps_qv", bufs=2, space="PSUM"))
            ps_kp = ctx.enter_context(tc.tile_pool(name="ps_k", bufs=2, space="PSUM"))
            ps_egp = ctx.enter_context(tc.tile_pool(name="ps_eg", bufs=2, space="PSUM"))
            ps_trp = ctx.enter_context(tc.tile_pool(name="ps_tr", bufs=2, space="PSUM"))

            def layer_norm(x_t, g_col, b_col, out_t, out_dt_tag, nb):
                """out = (x - mu) * rstd * g + b, via Square(bias=-mu, accum)."""
                nm = epi.tile([blk, 2], F32, tag=f"ln_nm{out_dt_tag}")
                nc.vector.tensor_reduce(
                    out=nm[:nb, 0:1], in_=x_t[:nb], axis=mybir.AxisListType.X,
                    op=OP.add, negate=True,
                )
                nc.vector.tensor_scalar_mul(
                    out=nm[:nb, 1:2], in0=nm[:nb, 0:1], scalar1=1.0 / D
                )
                sq = epi.tile([blk, D], F32, tag=f"ln_sq{out_dt_tag}")
                v2 = epi.tile([blk, 3], F32, tag=f"ln_v2{out_dt_tag}")
                nc.scalar.activation(
                    out=sq[:nb], in_=x_t[:nb], func=AF.Square,
                    bias=nm[:nb, 1:2], accum_out=v2[:nb, 0:1],
                )
                nc.scalar.activation(
                    out=v2[:nb, 1:2], in_=v2[:nb, 0:1], func=AF.Ln,
                    scale=1.0 / D, bias=EPS,
                )
                nc.scalar.activation(
                    out=v2[:nb, 2:3], in_=v2[:nb, 1:2], func=AF.Exp, scale=-0.5
                )
                t1 = epi.tile([blk, D], F32, tag=f"ln_t1{out_dt_tag}")
                nc.vector.scalar_tensor_tensor(
                    out=t1[:nb], in0=lnb[:nb, g_col * D : (g_col + 1) * D],
                    scalar=v2[:nb, 2:3], in1=zrow[:nb],
                    op0=OP.mult, op1=OP.add,
                )
                nc.vector.scalar_tensor_tensor(
                    out=out_t[:nb], in0=x_t[:nb], scalar=nm[:nb, 1:2],
                    in1=t1[:nb], op0=OP.add, op1=OP.mult,
                )
                nc.vector.tensor_add(
                    out=out_t[:nb], in0=out_t[:nb],
                    in1=lnb[:nb, b_col * D : (b_col + 1) * D],
                )

            for b in range(nblk):
                # --- loads ---
                fs16 = sb2.tile([128, CAP], F16, tag="fs16")
                ef16 = sb2.tile([128, CAP], F16, tag="ef16")
                if host_f16:
                    nc.sync.dma_start(
                        out=fs16[:], in_=fslotT_d[:, b * CAP : (b + 1) * CAP]
                    )
                    nc.sync.dma_start(
                        out=ef16[:], in_=eft_d[:, b * CAP : (b + 1) * CAP]
                    )
                else:
                    nc.gpsimd.dma_start(
                        out=fs16[:], in_=fslotT_d[:, b * CAP : (b + 1) * CAP]
                    )
                    nc.gpsimd.dma_start(
                        out=ef16[:], in_=eft_d[:, b * CAP : (b + 1) * CAP]
                    )
                ohT_t = sb2.tile([128, M, 128], F8, tag="oht")
                nc.sync.dma_start(out=ohT_t[:], in_=ohT_d[b])
                ohE_t = sb2.tile([128, M, 128], F8, tag="ohe")
                nc.sync.dma_start(out=ohE_t[:], in_=ohE_d[b])

                # --- eg = edge_feat @ [We|Wg], whole block in one PSUM bank ---
                egagg = ps_egp.tile([128, 512], F32, tag="egagg")
                ps_eg = egagg[:, 0 : M * 2 * H].rearrange(
                    "p (m c) -> p m c", c=2 * H
                )
                for j in range(M):
                    nc.tensor.matmul(
                        out=ps_eg[:, j, :],
                        lhsT=ef16[:, j * 128 : (j + 1) * 128],
                        rhs=weg16[:],
                        start=True, stop=True,
                    )
                eg16 = sb2.tile([128, M, 2 * H], F16, tag="eg16")
                nc.scalar.activation(out=eg16[:], in_=ps_eg, func=AF.Copy)

                # --- k expansion on PE, drained f16 (ACT/gpsimd split) ---
                k16 = sb2.tile([128, M, 128], F16, tag="k16")
                kgrps = []
                for gi, g0 in enumerate(range(0, M, 4)):
                    ng = min(4, M - g0)
                    ps_k = ps_kp.tile([128, 4, 128], F32, tag="k")
                    for jj in range(ng):
                        nc.tensor.matmul(
                            out=ps_k[:, jj, :],
                            lhsT=ohT_t[:blk, g0 + jj, :],
                            rhs=k_all[:, b, :],
                            start=True, stop=True,
                        )
                    kgrps.append((gi, g0, ng, ps_k))
                for gi, g0, ng, ps_k in kgrps:
                    if gi % 2 == 0:
                        nc.scalar.activation(
                            out=k16[:, g0 : g0 + ng, :], in_=ps_k[:, 0:ng, :],
                            func=AF.Copy,
                        )
                    else:
                        nc.vector.tensor_copy(
                            out=k16[:, g0 : g0 + ng, :], in_=ps_k[:, 0:ng, :]
                        )

                # --- q per edge (PSUM-resident, consumed by qk mul) ---
                qk16 = sb2a.tile([128, M, 128], F16, tag="qk")
                for g0 in range(0, M, 4):
                    ng = min(4, M - g0)
                    ps_q = ps_qvp.tile([128, 4, 128], F32, tag="qv")
                    for jj in range(ng):
                        nc.tensor.matmul(
                            out=ps_q[:, jj, :],
                            lhsT=fs16[:, (g0 + jj) * 128 : (g0 + jj + 1) * 128],
                            rhs=wqv16[:, 0:D],
                            start=True, stop=True,
                        )
                    nc.vector.tensor_mul(
                        out=qk16[:, g0 : g0 + ng, :],
                        in0=ps_q[:, 0:ng, :],
                        in1=k16[:, g0 : g0 + ng, :],
                    )

                # --- a = per-head dot (fold halves f16 2x, then reduce) ---
                qk8 = sb2a.tile([128, M * H, HD // 2], F16, tag="qk8")
                qkv = qk16[:].rearrange("p m (h f x) -> p (m h) f x", f=2, x=HD // 2)
                nc.vector.tensor_add(out=qk8[:], in0=qkv[:, :, 0, :], in1=qkv[:, :, 1, :])
                a16 = sb2a.tile([128, M * H], F16, tag="a")
                with nc.allow_low_precision(reason="8-term f16 head sum, |err|<0.1%"):
                    nc.vector.tensor_reduce(
                        out=a16[:],
                        in_=qk8[:],
                        axis=mybir.AxisListType.X,
                        op=OP.add,
                    )
                w16 = sb2a.tile([128, M, H], F16, tag="w")
                nc.vector.tensor_scalar(
                    out=w16[:].rearrange("p m h -> p (m h)"), in0=a16[:],
                    scalar1=5.0, scalar2=-5.0, op0=OP.min, op1=OP.max,
                )
                nc.vector.tensor_add(
                    out=w16[:], in0=w16[:], in1=eg16[:, :, 0:H]
                )
                # pu = [p | v*p*gate] per edge (bf16 for the agg matmul)
                pu = sb2a.tile([128, M, H + D], BF16, tag="pu")
                nc.scalar.activation(
                    out=pu[:, :, 0:H], in_=w16[:], func=AF.Exp, scale=4.0
                )
                # gates = sigmoid(eg[:, :, H:2H]) ; pg = p * gate
                sg16 = sb2a.tile([128, M, H], BF16, tag="sg")
                nc.scalar.activation(
                    out=sg16[:], in_=eg16[:, :, H : 2 * H], func=AF.Exp, scale=-1.0
                )
                nc.vector.tensor_scalar_add(out=sg16[:], in0=sg16[:], scalar1=1.0)
                gate16 = sb2a.tile([128, M, H], BF16, tag="gate")
                with nc.allow_low_precision(reason="sigmoid gate, |err|<0.4%"):
                    nc.vector.reciprocal(out=gate16[:], in_=sg16[:])
                pg = sb2a.tile([128, M, H], BF16, tag="pg")
                nc.vector.tensor_mul(out=pg[:], in0=pu[:, :, 0:H], in1=gate16[:])

                # --- v per edge + weighted aggregate ---
                assert M * 2 * H + (H + D) <= 512
                ps_agg = egagg[:, M * 2 * H : M * 2 * H + H + D]
                vgrps = []
                for g0 in range(0, M, 4):
                    ng = min(4, M - g0)
                    ps_v = ps_qvp.tile([128, 4, 128], F32, tag="qv")
                    for jj in range(ng):
                        nc.tensor.matmul(
                            out=ps_v[:, jj, :],
                            lhsT=fs16[:, (g0 + jj) * 128 : (g0 + jj + 1) * 128],
                            rhs=wqv16[:, D : 2 * D],
                            start=True, stop=True,
                        )
                    vgrps.append((g0, ng, ps_v))
                for g0, ng, ps_v in vgrps:
                    nc.vector.tensor_mul(
                        out=pu[:, g0 : g0 + ng, H : H + D].rearrange(
                            "p m (h x) -> p m h x", x=HD
                        ),
                        in0=ps_v[:, 0:ng, :].rearrange("p m (h x) -> p m h x", x=HD),
                        in1=pg[:, g0 : g0 + ng, :, None].to_broadcast(
                            [128, ng, H, HD]
                        ),
                    )
                    for jj in range(ng):
                        j = g0 + jj
                        nc.tensor.matmul(
                            out=ps_agg,
                            lhsT=ohE_t[:, j, :],
                            rhs=pu[:, j, :],
                            start=(j == 0),
                            stop=(j == M - 1),
                        )

                # --- node epilogue ---
                nb = blk
                dinv = epi.tile([blk, 2 * H], F32, tag="dinv")
                nc.vector.tensor_scalar_max(
                    out=dinv[:nb, 0:H], in0=ps_agg[:nb, 0:H], scalar1=1e-30
                )
                nc.vector.reciprocal(out=dinv[:nb, H : 2 * H], in_=dinv[:nb, 0:H])
                agg16 = epi.tile([blk, D], F16, tag="agg16")
                nc.vector.tensor_mul(
                    out=agg16[:nb].rearrange("p (h x) -> p h x", x=HD),
                    in0=ps_agg[:nb, H : H + D].rearrange("p (h x) -> p h x", x=HD),
                    in1=dinv[:nb, H : 2 * H, None].to_broadcast([nb, H, HD]),
                )

                trrs = ps_trp.tile([128, 512], F32, tag="trrs")
                ps_tr = trrs[:, 264:456].bitcast(F16).rearrange(
                    "p (s c) -> p s c", c=128
                )
                nc.tensor.transpose(
                    out=ps_tr[:, 0, 0:blk], in_=agg16[:nb], identity=ident16[:nb, :nb]
                )
                aggT16 = epi.tile([D, blk], F16, tag="aggT")
                nc.vector.tensor_copy(out=aggT16[:], in_=ps_tr[:, 0, 0:blk])

                # rsf: [0:D) rst | [D] gp_o | [D+1] gp_s | [D+2:2D+2) skip
                rsf = trrs[:blk, 0 : 2 * D + 2]
                nc.tensor.matmul(
                    out=rsf[:nb, 0 : D + 1], lhsT=aggT16[:, :nb], rhs=rhs_o16[:],
                    start=True, stop=True,
                )
                nc.tensor.matmul(
                    out=rsf[:nb, D + 1 : 2 * D + 2],
                    lhsT=featT16[:, b * blk : b * blk + nb],
                    rhs=rhs_s16[:],
                    start=True, stop=True,
                )
                sk32 = epi.tile([blk, D + 1], F32, tag="sk")
                nc.scalar.activation(
                    out=sk32[:nb], in_=rsf[:nb, D + 1 : 2 * D + 2], func=AF.Copy
                )
                gprc = epi.tile([blk, 3], F32, tag="gprc")
                nc.vector.tensor_add(
                    out=gprc[:nb, 0:1], in0=rsf[:nb, D : D + 1], in1=sk32[:nb, 0:1]
                )
                nc.scalar.activation(
                    out=gprc[:nb, 1:2], in_=gprc[:nb, 0:1], func=AF.Exp, scale=-1.0
                )
                nc.vector.tensor_scalar_add(
                    out=gprc[:nb, 1:2], in0=gprc[:nb, 1:2], scalar1=1.0
                )
                nc.vector.reciprocal(out=gprc[:nb, 2:3], in_=gprc[:nb, 1:2])
                diff = epi.tile([blk, D], F32, tag="diff")
                nc.vector.tensor_sub(
                    out=diff[:nb], in0=rsf[:nb, 0:D], in1=sk32[:nb, 1 : D + 1]
                )
                mix = epi.tile([blk, D], F32, tag="mix")
                nc.vector.scalar_tensor_tensor(
                    out=mix[:nb], in0=diff[:nb], scalar=gprc[:nb, 2:3],
                    in1=sk32[:nb, 1 : D + 1],
                    op0=OP.mult, op1=OP.add,
                )

                h32 = epi.tile([blk, D], F32, tag="h")
                layer_norm(mix, 0, 1, h32, "1", nb)
                l216 = epi.tile([blk, D], F16, tag="l2")
                layer_norm(h32, 2, 3, l216, "2", nb)

                nc.tensor.transpose(
                    out=ps_tr[:, 1, 0:blk], in_=l216[:nb], identity=ident16[:nb, :nb]
                )
                l2T16 = epi.tile([D, blk], F16, tag="l2T")
                nc.vector.tensor_copy(out=l2T16[:], in_=ps_tr[:, 1, 0:blk])
                nc.tensor.matmul(
                    out=rsf[:nb, D + 2 : 2 * D + 2], lhsT=l2T16[:, :nb], rhs=w1_16[:],
                    start=True, stop=True,
                )
                r16 = epi.tile([blk, D], F16, tag="r")
                nc.scalar.activation(
                    out=r16[:nb], in_=rsf[:nb, D + 2 : 2 * D + 2], func=AF.Relu
                )
                nc.tensor.transpose(
                    out=ps_tr[:, 2, 0:blk], in_=r16[:nb], identity=ident16[:nb, :nb]
                )
                rT16 = epi.tile([D, blk], F16, tag="rT")
                nc.vector.tensor_copy(out=rT16[:], in_=ps_tr[:, 2, 0:blk])
                nc.tensor.matmul(
                    out=rsf[:nb, 0:D], lhsT=rT16[:, :nb], rhs=w2_16[:],
                    start=True, stop=True,
                )
                outb = epi.tile([blk, D], F32, tag="outb")
                nc.vector.tensor_add(
                    out=outb[:nb], in0=h32[:nb], in1=rsf[:nb, 0:D]
                )
                nc.scalar.dma_start(
                    out=out_d[b * blk : b * blk + nb, :], in_=outb[:nb]
                )

    nc.compile()
    return nc


def compute_layout(inputs, base):
    """Decide the data-dependent static block capacity M (tiles per block)."""
    cores, npc, nblk, blk = base["cores"], base["npc"], base["nblk"], base["blk"]
    nblk_g = cores * nblk

    src = np.asarray(inputs["src"]).astype(np.int64)
    dst = np.asarray(inputs["dst"]).astype(np.int64)
    gb_all = dst // blk
    order = np.lexsort((src, gb_all))  # by block, then src
    ds = dst[order]
    ss = src[order]
    gb = gb_all[order]

    counts = np.bincount(gb, minlength=nblk_g)
    M = max(2, int(np.ceil(counts.max() / 128)))

    starts = np.zeros(nblk_g + 1, dtype=np.int64)
    np.cumsum(counts, out=starts[1:])
    pos = np.arange(len(ds)) - starts[gb]
    slot = gb * (M * 128) + pos

    layout = dict(order=order, ds=ds, ss=ss, gb=gb, slot=slot)
    cfg = dict(base, M=M, host_f16=HOST_F16)
    return cfg, layout


def shard_inputs(inputs, cfg, layout):
    """Host-side layout only (sort/pad/transpose/index; dtype staging)."""
    cores = cfg["cores"]
    npc = cfg["npc"]
    nblk = cfg["nblk"]
    blk = cfg["blk"]
    M = cfg["M"]
    CAP = M * 128
    nblk_g = cores * nblk
    np_big = np.float16 if cfg["host_f16"] else np.float32

    ds, ss, slot = layout["ds"], layout["ss"], layout["slot"]
    gb = layout["gb"]
    edge_feat = np.asarray(inputs["edge_feat"])
    feat = np.asarray(inputs["feat"])

    total = nblk_g * CAP
    dstloc = np.full(total, blk, dtype=np.int64)
    dstloc[slot] = ds - gb * blk

    ef_pad = np.zeros((total, D), dtype=np_big)
    ef_pad[slot] = edge_feat[layout["order"]].astype(np_big)
    fs_pad = np.zeros((total, D), dtype=np_big)
    fs_pad[slot] = feat[ss].astype(np_big)

    f8 = mybir.dt.np(F8)
    sb_ = np.arange(total) % CAP
    gb_s = np.arange(total) // CAP

    # transposed one-hot (k-expansion lhsT): ohT[b, n, j, p] = 1 iff
    # dst_local(edge at slot j*128+p of block b) == n
    ohT = np.zeros(nblk_g * 128 * CAP, dtype=f8)
    oh_idx = ((gb_s * 128 + dstloc) * (CAP // 128) + sb_ // 128) * 128 + sb_ % 128
    ohT[oh_idx] = 1.0
    ohT = ohT.reshape(nblk_g, 128, CAP // 128, 128)

    # edge-major one-hot (agg lhsT): ohE[b, p, j, n] = 1 iff dst_local == n
    ohE = np.zeros(nblk_g * CAP * 128, dtype=f8)
    ohE_idx = ((gb_s * CAP + sb_) * 128) + dstloc
    ohE[ohE_idx] = 1.0
    ohE = ohE.reshape(nblk_g, CAP // 128, 128, 128).transpose(0, 2, 1, 3)
    ohE = np.ascontiguousarray(ohE)

    per_core = nblk * CAP
    in_maps = []
    for c_i in range(cores):
        bsl = slice(c_i * nblk, (c_i + 1) * nblk)
        sl = slice(c_i * per_core, (c_i + 1) * per_core)
        m = {
            "eft": np.ascontiguousarray(ef_pad[sl].T),
            "fslotT": np.ascontiguousarray(fs_pad[sl].T),
            "ohT": np.ascontiguousarray(ohT[bsl]),
            "ohE": np.ascontiguousarray(ohE[bsl]),
            "featT": np.ascontiguousarray(
                feat[c_i * npc : (c_i + 1) * npc].astype(np_big).T
            ),
            "WoT": np.ascontiguousarray(np.asarray(inputs["Wo"]).T),
            "WskipT": np.ascontiguousarray(np.asarray(inputs["Wskip"]).T),
        }
        for name in ("Wq", "Wk", "Wv", "Wo", "Wskip", "W1", "W2", "We", "Wg",
                     "Wgate", "ln1_g", "ln1_b", "ln2_g", "ln2_b"):
            m[name] = np.ascontiguousarray(np.asarray(inputs[name]))
        in_maps.append(m)
    return in_maps


_cache = {}


def _get_program(cfg):
    key = (cfg["cores"], cfg["M"], cfg["host_f16"])
    if key not in _cache:
        _cache[key] = build_program(cfg)
    return _cache[key]


def full_base():
    return dict(cores=CORES, n_nodes=N_NODES, npc=NPC, nblk=NBLK, blk=BLK)


def _ensure_ntff_hook():
    """The agent image's antenv lacks axon_hooks; synthesize it from the
    boot module's ctypes NTFF profiler so trace=True can capture timings."""
    import types

    if "antenv.axon_hooks" in sys.modules:
        return
    try:
        sys.path.insert(0, "/root/.axon_site")
        from trn_agent_boot.trn_boot import _ntff_profile_via_ctypes

        hook = _ntff_profile_via_ctypes("/opt/axon/libaxon_pjrt.so")
        mod = types.ModuleType("antenv.axon_hooks")
        mod.get_axon_ntff_profile_hook = lambda: hook
        mod.set_axon_ntff_profile_hook = lambda h: None
        sys.modules["antenv.axon_hooks"] = mod
    except Exception as e:  # degrade to untimed run
        print(f"ntff hook setup failed: {e}")


def run(inputs, trace=False, tmpdir=None, trace_cores=None):
    if trace:
        _ensure_ntff_hook()
    cfg, layout = compute_layout(inputs, full_base())
    nc = _get_program(cfg)
    in_maps = shard_inputs(inputs, cfg, layout)
    res = bass_utils.run_bass_kernel_spmd(
        nc,
        in_maps,
        core_ids=list(range(cfg["cores"])),
        trace=trace,
        tmpdir=tmpdir,
        trace_cores=trace_cores,
    )
    out = np.concatenate([res.results[c]["out"] for c in range(cfg["cores"])], axis=0)
    return out, res


def kernel(**inputs):
    out, _ = run(inputs)
    return out


# revision 30
# speedup vs baseline: 1.0195x; 1.0195x over previous
"""Trainium2 Bass kernel for GAT-style GNN message passing (edge softmax).

Contract: kernel(**inputs) takes FULL unsharded numpy inputs, distributes
across 8 NeuronCores internally, returns FULL output.

Sharding: edges sorted by dst and partitioned by dst range (6250 nodes per
core) -> every per-destination segment reduction is core-local. Node
features/weights replicated.

v2 design notes (vs v1 baseline):
- All matmuls use f16/bf16/f8 operands (fp32 matmul is 4 cyc/row on PE).
- edge_feat projections (e_bias/gates) computed in the main block loop
  (no separate stage + DRAM round trip).
- f32->f16 conversion happens inside the DMA (SWDGE cast) - no on-chip
  cast instructions.
- q and v are consumed directly from PSUM by the DVE ops that need them
  (the op IS the drain); k is drained to f16 SBUF split ACT/gpsimd.
- Edge-major one-hot (agg lhsT) shipped from host as f8 instead of an
  on-device is_equal build.
- k for own nodes kept resident in SBUF (no DRAM round trip).
- LayerNorm restructured: Square(bias=-mu, accum) + folded affine.
"""

import os
import sys

sys.path.insert(0, "/opt/trn_rl_repo")

import numpy as np

import concourse.bass as bass
import concourse.mybir as mybir
import concourse.tile as tile
from concourse import bacc
from concourse import bass_utils
from concourse.masks import make_identity

F32 = mybir.dt.float32
F16 = mybir.dt.float16
BF16 = mybir.dt.bfloat16
F8 = mybir.dt.float8e4
I32 = mybir.dt.int32
AF = mybir.ActivationFunctionType
OP = mybir.AluOpType

D = 128
H = 8
HD = 16
EPS = 1e-5

N_NODES = 50000
N_EDGES = 800000
CORES = 8
NPC = N_NODES // CORES      # nodes per core = 6250
BLK = 125                   # dst nodes per block
NBLK = NPC // BLK           # 50 blocks per core

# If "1": host stages the big per-edge tables (edge_feat, feat[src]) in
# fp16 (same values the device math would see after its own cast).
HOST_F16 = os.environ.get("KERNEL_HOST_F16", "1") == "1"


def _force_act_set():
    """Pin every ACTIVATE to the natural_log_exp_and_others table so the
    kernel pays one ACT_TABLE_LOAD instead of hundreds."""
    from concourse import hw_specs

    if getattr(bacc, "_act_set_forced", False):
        return
    real = hw_specs.get_activation_tables

    def patched(arch):
        t = dict(real(arch))
        keep = "natural_log_exp_and_others"
        return {name: (fns if name == keep else set()) for name, fns in t.items()}

    bacc.get_activation_tables = patched
    bacc._act_set_forced = True


def build_program(cfg):
    _force_act_set()
    cores = cfg["cores"]
    npc = cfg["npc"]
    nblk = cfg["nblk"]
    blk = cfg["blk"]
    M = cfg["M"]
    CAP = M * 128
    EPC = nblk * CAP
    host_f16 = cfg["host_f16"]
    big_dt = F16 if host_f16 else F32

    nc = bacc.Bacc(
        "TRN2", target_bir_lowering=False, debug=False, num_devices=cores
    )

    # ---- I/O ----
    eft_d = nc.dram_tensor("eft", [D, EPC], big_dt, kind="ExternalInput").ap()
    fslotT_d = nc.dram_tensor("fslotT", [D, EPC], big_dt, kind="ExternalInput").ap()
    ohT_d = nc.dram_tensor("ohT", [nblk, 128, M, 128], F8, kind="ExternalInput").ap()
    ohE_d = nc.dram_tensor("ohE", [nblk, 128, M, 128], F8, kind="ExternalInput").ap()
    featT_d = nc.dram_tensor("featT", [D, npc], big_dt, kind="ExternalInput").ap()
    w_in = {}
    for name in ("Wq", "Wk", "Wv", "Wo", "Wskip", "W1", "W2", "WoT", "WskipT"):
        w_in[name] = nc.dram_tensor(name, [D, D], F32, kind="ExternalInput").ap()
    w_in["We"] = nc.dram_tensor("We", [D, H], F32, kind="ExternalInput").ap()
    w_in["Wg"] = nc.dram_tensor("Wg", [D, H], F32, kind="ExternalInput").ap()
    w_in["Wgate"] = nc.dram_tensor("Wgate", [3 * D, 1], F32, kind="ExternalInput").ap()
    for name in ("ln1_g", "ln1_b", "ln2_g", "ln2_b"):
        w_in[name] = nc.dram_tensor(name, [D], F32, kind="ExternalInput").ap()
    out_d = nc.dram_tensor("out", [npc, D], F16, kind="ExternalOutput").ap()

    with tile.TileContext(nc) as tc:
        import contextlib

        ctx = contextlib.ExitStack()
        with ctx:
            consts = ctx.enter_context(tc.tile_pool(name="consts", bufs=1))

            # ---------- setup ----------
            ident16 = consts.tile([128, 128], F16)
            make_identity(nc, ident16[:])

            ones_row = consts.tile([1, 128], F32)
            nc.vector.memset(ones_row[:], 1.0)

            zrow = consts.tile([128, D], F32)
            nc.vector.memset(zrow[:], 0.0)

            const2 = consts.tile([128, 2], F32)
            nc.vector.memset(const2[:, 0:1], 0.0)
            nc.vector.memset(const2[:, 1:2], EPS)
            nc.const_aps.aps[(F32, 0.0)] = const2[:, 0:1]
            nc.const_aps.aps[(F32, EPS)] = const2[:, 1:2]

            lnrow = consts.tile([1, 4 * D], F32)
            for i, name in enumerate(("ln1_g", "ln1_b", "ln2_g", "ln2_b")):
                nc.sync.dma_start(
                    out=lnrow[:, i * D : (i + 1) * D], in_=w_in[name][None, :]
                )
            lnb = consts.tile([128, 4 * D], F32)

            wqv32 = consts.tile([D, 2 * D], F32)
            nc.sync.dma_start(out=wqv32[:, 0:D], in_=w_in["Wq"][:])
            nc.sync.dma_start(out=wqv32[:, D : 2 * D], in_=w_in["Wv"][:])
            wqv16 = consts.tile([D, 2 * D], F16)
            nc.vector.tensor_copy(out=wqv16[:], in_=wqv32[:])
            wk16 = consts.tile([D, D], F16)
            nc.gpsimd.dma_start(out=wk16[:], in_=w_in["Wk"][:])
            weg32 = consts.tile([D, 2 * H], F32)
            nc.sync.dma_start(out=weg32[:, 0:H], in_=w_in["We"][:])
            nc.sync.dma_start(out=weg32[:, H : 2 * H], in_=w_in["Wg"][:])
            weg16 = consts.tile([D, 2 * H], F16)
            nc.vector.tensor_copy(out=weg16[:], in_=weg32[:])
            w1_16 = consts.tile([D, D], F16)
            nc.gpsimd.dma_start(out=w1_16[:], in_=w_in["W1"][:])
            w2_16 = consts.tile([D, D], F16)
            nc.gpsimd.dma_start(out=w2_16[:], in_=w_in["W2"][:])

            # gate vector folding: gate_pre = agg@(Wo@A) + feat@(Wskip@B)
            wg3 = consts.tile([128, 3], F32)
            nc.sync.dma_start(
                out=wg3[:], in_=w_in["Wgate"].rearrange("(t p) c -> p (t c)", p=128)
            )
            ab = consts.tile([128, 2], F32)
            nc.vector.tensor_add(out=ab[:, 0:1], in0=wg3[:, 0:1], in1=wg3[:, 2:3])
            nc.vector.tensor_sub(out=ab[:, 1:2], in0=wg3[:, 1:2], in1=wg3[:, 2:3])

            wot_s = consts.tile([D, D], F32)
            nc.sync.dma_start(out=wot_s[:], in_=w_in["WoT"][:])
            wskipt_s = consts.tile([D, D], F32)
            nc.sync.dma_start(out=wskipt_s[:], in_=w_in["WskipT"][:])

            rhs_o32 = consts.tile([D, D + 1], F32)
            nc.sync.dma_start(out=rhs_o32[:, 0:D], in_=w_in["Wo"][:])
            rhs_s32 = consts.tile([D, D + 1], F32)
            nc.sync.dma_start(out=rhs_s32[:, 1 : D + 1], in_=w_in["Wskip"][:])

            with tc.tile_pool(name="psum_setup", bufs=1, space="PSUM") as pss:
                ps_ln = pss.tile([128, 4 * D], F32, tag="ln")
                nc.tensor.matmul(
                    out=ps_ln[:], lhsT=ones_row[:], rhs=lnrow[:], start=True, stop=True
                )
                nc.vector.tensor_copy(out=lnb[:], in_=ps_ln[:])

                ps_c = pss.tile([128, 2], F32, tag="c")
                nc.tensor.matmul(
                    out=ps_c[:, 0:1], lhsT=wot_s[:], rhs=ab[:, 0:1],
                    start=True, stop=True,
                )
                nc.tensor.matmul(
                    out=ps_c[:, 1:2], lhsT=wskipt_s[:], rhs=ab[:, 1:2],
                    start=True, stop=True,
                )
                nc.vector.tensor_copy(out=rhs_o32[:, D : D + 1], in_=ps_c[:, 0:1])
                nc.vector.tensor_copy(out=rhs_s32[:, 0:1], in_=ps_c[:, 1:2])

            rhs_o16 = consts.tile([D, D + 1], F16)
            nc.vector.tensor_copy(out=rhs_o16[:], in_=rhs_o32[:])
            rhs_s16 = consts.tile([D, D + 1], F16)
            nc.vector.tensor_copy(out=rhs_s16[:], in_=rhs_s32[:])

            featT16 = consts.tile([D, npc], F16)
            if host_f16:
                nc.sync.dma_start(out=featT16[:], in_=featT_d[:])
            else:
                nc.gpsimd.dma_start(out=featT16[:], in_=featT_d[:])

            # ---------- stage 1: k for own nodes, SBUF-resident ----------
            # k_all[n_local, b, :] = (feat @ Wk)[b*blk + n_local, :]
            k_all = consts.tile([blk, nblk, D], F16)
            with tc.tile_pool(name="k1_ps", bufs=2, space="PSUM") as kps:
                for g0 in range(0, nblk, 4):
                    ng = min(4, nblk - g0)
                    ps_k1 = kps.tile([blk, 4, D], F32, tag="k1")
                    for j in range(ng):
                        b = g0 + j
                        nc.tensor.matmul(
                            out=ps_k1[:, j, :],
                            lhsT=featT16[:, b * blk : (b + 1) * blk],
                            rhs=wk16[:],
                            start=True, stop=True,
                        )
                    nc.scalar.activation(
                        out=k_all[:, g0 : g0 + ng, :], in_=ps_k1[:, 0:ng, :],
                        func=AF.Copy,
                    )

            # ---------- main loop ----------
            sb2 = ctx.enter_context(tc.tile_pool(name="p2_sb", bufs=3))
            sb2a = ctx.enter_context(tc.tile_pool(name="p2_sba", bufs=2))
            epi = ctx.enter_context(tc.tile_pool(name="epi_sb", bufs=2))
            # PSUM bank packing (8 banks x 2KB): q/v share one rotating tag
            # (2 banks), k 2 banks, eg+agg packed in one tile (2 banks),
            # tr 1 bank, rs 1 bank.
            ps_qvp = ctx.enter_context(tc.tile_pool(name="ps_qv", bufs=2, space="PSUM"))
            ps_kp = ctx.enter_context(tc.tile_pool(name="ps_k", bufs=2, space="PSUM"))
            ps_egp = ctx.enter_context(tc.tile_pool(name="ps_eg", bufs=2, space="PSUM"))
            ps_trp = ctx.enter_context(tc.tile_pool(name="ps_tr", bufs=2, space="PSUM"))

            def layer_norm(x_t, g_col, b_col, out_t, out_dt_tag, nb):
                """out = (x - mu) * rstd * g + b, via Square(bias=-mu, accum)."""
                nm = epi.tile([blk, 2], F32, tag=f"ln_nm{out_dt_tag}")
                nc.vector.tensor_reduce(
                    out=nm[:nb, 0:1], in_=x_t[:nb], axis=mybir.AxisListType.X,
                    op=OP.add, negate=True,
                )
                nc.vector.tensor_scalar_mul(
                    out=nm[:nb, 1:2], in0=nm[:nb, 0:1], scalar1=1.0 / D
                )
                sq = epi.tile([blk, D], F32, tag=f"ln_sq{out_dt_tag}")
                v2 = epi.tile([blk, 3], F32, tag=f"ln_v2{out_dt_tag}")
                nc.scalar.activation(
                    out=sq[:nb], in_=x_t[:nb], func=AF.Square,
                    bias=nm[:nb, 1:2], accum_out=v2[:nb, 0:1],
                )
                nc.scalar.activation(
                    out=v2[:nb, 1:2], in_=v2[:nb, 0:1], func=AF.Ln,
                    scale=1.0 / D, bias=EPS,
                )
                nc.scalar.activation(
                    out=v2[:nb, 2:3], in_=v2[:nb, 1:2], func=AF.Exp, scale=-0.5
                )
                t1 = epi.tile([blk, D], F32, tag=f"ln_t1{out_dt_tag}")
                nc.vector.scalar_tensor_tensor(
                    out=t1[:nb], in0=lnb[:nb, g_col * D : (g_col + 1) * D],
                    scalar=v2[:nb, 2:3], in1=zrow[:nb],
                    op0=OP.mult, op1=OP.add,
                )
                nc.vector.scalar_tensor_tensor(
                    out=out_t[:nb], in0=x_t[:nb], scalar=nm[:nb, 1:2],
                    in1=t1[:nb], op0=OP.add, op1=OP.mult,
                )
                nc.vector.tensor_add(
                    out=out_t[:nb], in0=out_t[:nb],
                    in1=lnb[:nb, b_col * D : (b_col + 1) * D],
                )

            for b in range(nblk):
                # --- loads ---
                fs16 = sb2.tile([128, CAP], F16, tag="fs16")
                ef16 = sb2.tile([128, CAP], F16, tag="ef16")
                if host_f16:
                    nc.sync.dma_start(
                        out=fs16[:], in_=fslotT_d[:, b * CAP : (b + 1) * CAP]
                    )
                    nc.sync.dma_start(
                        out=ef16[:], in_=eft_d[:, b * CAP : (b + 1) * CAP]
                    )
                else:
                    nc.gpsimd.dma_start(
                        out=fs16[:], in_=fslotT_d[:, b * CAP : (b + 1) * CAP]
                    )
                    nc.gpsimd.dma_start(
                        out=ef16[:], in_=eft_d[:, b * CAP : (b + 1) * CAP]
                    )
                ohT_t = sb2.tile([128, M, 128], F8, tag="oht")
                nc.sync.dma_start(out=ohT_t[:], in_=ohT_d[b])
                ohE_t = sb2.tile([128, M, 128], F8, tag="ohe")
                nc.sync.dma_start(out=ohE_t[:], in_=ohE_d[b])

                # --- eg = edge_feat @ [We|Wg], whole block in one PSUM bank ---
                egagg = ps_egp.tile([128, 512], F32, tag="egagg")
                ps_eg = egagg[:, 0 : M * 2 * H].rearrange(
                    "p (m c) -> p m c", c=2 * H
                )
                for j in range(M):
                    nc.tensor.matmul(
                        out=ps_eg[:, j, :],
                        lhsT=ef16[:, j * 128 : (j + 1) * 128],
                        rhs=weg16[:],
                        start=True, stop=True,
                    )
                eg16 = sb2.tile([128, M, 2 * H], F16, tag="eg16")
                nc.scalar.activation(out=eg16[:], in_=ps_eg, func=AF.Copy)

                # --- k expansion on PE, drained f16 (ACT/gpsimd split) ---
                k16 = sb2.tile([128, M, 128], F16, tag="k16")
                kgrps = []
                for gi, g0 in enumerate(range(0, M, 4)):
                    ng = min(4, M - g0)
                    ps_k = ps_kp.tile([128, 4, 128], F32, tag="k")
                    for jj in range(ng):
                        nc.tensor.matmul(
                            out=ps_k[:, jj, :],
                            lhsT=ohT_t[:blk, g0 + jj, :],
                            rhs=k_all[:, b, :],
                            start=True, stop=True,
                        )
                    kgrps.append((gi, g0, ng, ps_k))
                for gi, g0, ng, ps_k in kgrps:
                    if gi % 2 == 0:
                        nc.scalar.activation(
                            out=k16[:, g0 : g0 + ng, :], in_=ps_k[:, 0:ng, :],
                            func=AF.Copy,
                        )
                    else:
                        nc.vector.tensor_copy(
                            out=k16[:, g0 : g0 + ng, :], in_=ps_k[:, 0:ng, :]
                        )

                # --- q per edge (PSUM-resident, consumed by qk mul) ---
                qk16 = sb2a.tile([128, M, 128], F16, tag="qk")
                for g0 in range(0, M, 4):
                    ng = min(4, M - g0)
                    ps_q = ps_qvp.tile([128, 4, 128], F32, tag="qv")
                    for jj in range(ng):
                        nc.tensor.matmul(
                            out=ps_q[:, jj, :],
                            lhsT=fs16[:, (g0 + jj) * 128 : (g0 + jj + 1) * 128],
                            rhs=wqv16[:, 0:D],
                            start=True, stop=True,
                        )
                    nc.vector.tensor_mul(
                        out=qk16[:, g0 : g0 + ng, :],
                        in0=ps_q[:, 0:ng, :],
                        in1=k16[:, g0 : g0 + ng, :],
                    )

                # --- a = per-head dot (fold halves f16 2x, then reduce) ---
                qk8 = sb2a.tile([128, M * H, HD // 2], F16, tag="qk8")
                qkv = qk16[:].rearrange("p m (h f x) -> p (m h) f x", f=2, x=HD // 2)
                nc.vector.tensor_add(out=qk8[:], in0=qkv[:, :, 0, :], in1=qkv[:, :, 1, :])
                qk4 = sb2a.tile([128, M * H, HD // 4], F16, tag="qk4")
                qv2 = qk8[:].rearrange("p m (f x) -> p m f x", f=2, x=HD // 4)
                nc.vector.tensor_add(
                    out=qk4[:], in0=qv2[:, :, 0, :], in1=qv2[:, :, 1, :]
                )
                a32 = sb2a.tile([128, M * H], F32, tag="a")
                nc.vector.tensor_reduce(
                    out=a32[:],
                    in_=qk4[:],
                    axis=mybir.AxisListType.X,
                    op=OP.add,
                )
                w16 = sb2a.tile([128, M, H], F16, tag="w")
                nc.vector.tensor_scalar(
                    out=w16[:].rearrange("p m h -> p (m h)"), in0=a32[:],
                    scalar1=5.0, scalar2=-5.0, op0=OP.min, op1=OP.max,
                )
                nc.vector.tensor_add(
                    out=w16[:], in0=w16[:], in1=eg16[:, :, 0:H]
                )
                # pu = [p | v*p*gate] per edge (bf16 for the agg matmul)
                pu = sb2a.tile([128, M, H + D], BF16, tag="pu")
                nc.scalar.activation(
                    out=pu[:, :, 0:H], in_=w16[:], func=AF.Exp, scale=4.0
                )
                # gates = sigmoid(eg[:, :, H:2H]) ; pg = p * gate
                sg16 = sb2a.tile([128, M, H], BF16, tag="sg")
                nc.scalar.activation(
                    out=sg16[:], in_=eg16[:, :, H : 2 * H], func=AF.Exp, scale=-1.0
                )
                nc.vector.tensor_scalar_add(out=sg16[:], in0=sg16[:], scalar1=1.0)
                gate16 = sb2a.tile([128, M, H], BF16, tag="gate")
                with nc.allow_low_precision(reason="sigmoid gate, |err|<0.4%"):
                    nc.vector.reciprocal(out=gate16[:], in_=sg16[:])
                pg = sb2a.tile([128, M, H], BF16, tag="pg")
                nc.vector.tensor_mul(out=pg[:], in0=pu[:, :, 0:H], in1=gate16[:])

                # --- v per edge + weighted aggregate ---
                assert M * 2 * H + (H + D) <= 512
                ps_agg = egagg[:, M * 2 * H : M * 2 * H + H + D]
                vgrps = []
                for g0 in range(0, M, 4):
                    ng = min(4, M - g0)
                    ps_v = ps_qvp.tile([128, 4, 128], F32, tag="qv")
                    for jj in range(ng):
                        nc.tensor.matmul(
                            out=ps_v[:, jj, :],
                            lhsT=fs16[:, (g0 + jj) * 128 : (g0 + jj + 1) * 128],
                            rhs=wqv16[:, D : 2 * D],
                            start=True, stop=True,
                        )
                    vgrps.append((g0, ng, ps_v))
                for g0, ng, ps_v in vgrps:
                    nc.vector.tensor_mul(
                        out=pu[:, g0 : g0 + ng, H : H + D].rearrange(
                            "p m (h x) -> p m h x", x=HD
                        ),
                        in0=ps_v[:, 0:ng, :].rearrange("p m (h x) -> p m h x", x=HD),
                        in1=pg[:, g0 : g0 + ng, :, None].to_broadcast(
                            [128, ng, H, HD]
                        ),
                    )
                    for jj in range(ng):
                        j = g0 + jj
                        nc.tensor.matmul(
                            out=ps_agg,
                            lhsT=ohE_t[:, j, :],
                            rhs=pu[:, j, :],
                            start=(j == 0),
                            stop=(j == M - 1),
                        )

                # --- node epilogue ---
                nb = blk
                dinv = epi.tile([blk, 2 * H], F32, tag="dinv")
                nc.vector.tensor_scalar_max(
                    out=dinv[:nb, 0:H], in0=ps_agg[:nb, 0:H], scalar1=1e-30
                )
                nc.vector.reciprocal(out=dinv[:nb, H : 2 * H], in_=dinv[:nb, 0:H])
                agg16 = epi.tile([blk, D], F16, tag="agg16")
                nc.vector.tensor_mul(
                    out=agg16[:nb].rearrange("p (h x) -> p h x", x=HD),
                    in0=ps_agg[:nb, H : H + D].rearrange("p (h x) -> p h x", x=HD),
                    in1=dinv[:nb, H : 2 * H, None].to_broadcast([nb, H, HD]),
                )

                trrs = ps_trp.tile([128, 512], F32, tag="trrs")
                ps_tr = trrs[:, 264:456].bitcast(F16).rearrange(
                    "p (s c) -> p s c", c=128
                )
                nc.tensor.transpose(
                    out=ps_tr[:, 0, 0:blk], in_=agg16[:nb], identity=ident16[:nb, :nb]
                )
                aggT16 = epi.tile([D, blk], F16, tag="aggT")
                nc.vector.tensor_copy(out=aggT16[:], in_=ps_tr[:, 0, 0:blk])

                # rsf: [0:D) rst | [D] gp_o | [D+1] gp_s | [D+2:2D+2) skip
                rsf = trrs[:blk, 0 : 2 * D + 2]
                nc.tensor.matmul(
                    out=rsf[:nb, 0 : D + 1], lhsT=aggT16[:, :nb], rhs=rhs_o16[:],
                    start=True, stop=True,
                )
                nc.tensor.matmul(
                    out=rsf[:nb, D + 1 : 2 * D + 2],
                    lhsT=featT16[:, b * blk : b * blk + nb],
                    rhs=rhs_s16[:],
                    start=True, stop=True,
                )
                sk32 = epi.tile([blk, D + 1], F32, tag="sk")
                nc.scalar.activation(
                    out=sk32[:nb], in_=rsf[:nb, D + 1 : 2 * D + 2], func=AF.Copy
                )
                gprc = epi.tile([blk, 3], F32, tag="gprc")
                nc.vector.tensor_add(
                    out=gprc[:nb, 0:1], in0=rsf[:nb, D : D + 1], in1=sk32[:nb, 0:1]
                )
                nc.scalar.activation(
                    out=gprc[:nb, 1:2], in_=gprc[:nb, 0:1], func=AF.Exp, scale=-1.0
                )
                nc.vector.tensor_scalar_add(
                    out=gprc[:nb, 1:2], in0=gprc[:nb, 1:2], scalar1=1.0
                )
                nc.vector.reciprocal(out=gprc[:nb, 2:3], in_=gprc[:nb, 1:2])
                diff = epi.tile([blk, D], F32, tag="diff")
                nc.vector.tensor_sub(
                    out=diff[:nb], in0=rsf[:nb, 0:D], in1=sk32[:nb, 1 : D + 1]
                )
                mix = epi.tile([blk, D], F32, tag="mix")
                nc.vector.scalar_tensor_tensor(
                    out=mix[:nb], in0=diff[:nb], scalar=gprc[:nb, 2:3],
                    in1=sk32[:nb, 1 : D + 1],
                    op0=OP.mult, op1=OP.add,
                )

                h32 = epi.tile([blk, D], F32, tag="h")
                layer_norm(mix, 0, 1, h32, "1", nb)
                l216 = epi.tile([blk, D], F16, tag="l2")
                layer_norm(h32, 2, 3, l216, "2", nb)

                nc.tensor.transpose(
                    out=ps_tr[:, 1, 0:blk], in_=l216[:nb], identity=ident16[:nb, :nb]
                )
                l2T16 = epi.tile([D, blk], F16, tag="l2T")
                nc.vector.tensor_copy(out=l2T16[:], in_=ps_tr[:, 1, 0:blk])
                nc.tensor.matmul(
                    out=rsf[:nb, D + 2 : 2 * D + 2], lhsT=l2T16[:, :nb], rhs=w1_16[:],
                    start=True, stop=True,
                )
                r16 = epi.tile([blk, D], F16, tag="r")
                nc.scalar.activation(
                    out=r16[:nb], in_=rsf[:nb, D + 2 : 2 * D + 2], func=AF.Relu
                )
                nc.tensor.transpose(
                    out=ps_tr[:, 2, 0:blk], in_=r16[:nb], identity=ident16[:nb, :nb]
                )
                rT16 = epi.tile([D, blk], F16, tag="rT")
                nc.vector.tensor_copy(out=rT16[:], in_=ps_tr[:, 2, 0:blk])
                nc.tensor.matmul(
                    out=rsf[:nb, 0:D], lhsT=rT16[:, :nb], rhs=w2_16[:],
                    start=True, stop=True,
                )
                outb = epi.tile([blk, D], F16, tag="outb")
                nc.vector.tensor_add(
                    out=outb[:nb], in0=h32[:nb], in1=rsf[:nb, 0:D]
                )
                nc.scalar.dma_start(
                    out=out_d[b * blk : b * blk + nb, :], in_=outb[:nb]
                )

    nc.compile()
    return nc


def compute_layout(inputs, base):
    """Decide the data-dependent static block capacity M (tiles per block)."""
    cores, npc, nblk, blk = base["cores"], base["npc"], base["nblk"], base["blk"]
    nblk_g = cores * nblk

    src = np.asarray(inputs["src"]).astype(np.int64)
    dst = np.asarray(inputs["dst"]).astype(np.int64)
    gb_all = dst // blk
    order = np.lexsort((src, gb_all))  # by block, then src
    ds = dst[order]
    ss = src[order]
    gb = gb_all[order]

    counts = np.bincount(gb, minlength=nblk_g)
    M = max(2, int(np.ceil(counts.max() / 128)))

    starts = np.zeros(nblk_g + 1, dtype=np.int64)
    np.cumsum(counts, out=starts[1:])
    pos = np.arange(len(ds)) - starts[gb]
    slot = gb * (M * 128) + pos

    layout = dict(order=order, ds=ds, ss=ss, gb=gb, slot=slot)
    cfg = dict(base, M=M, host_f16=HOST_F16)
    return cfg, layout


def shard_inputs(inputs, cfg, layout):
    """Host-side layout only (sort/pad/transpose/index; dtype staging)."""
    cores = cfg["cores"]
    npc = cfg["npc"]
    nblk = cfg["nblk"]
    blk = cfg["blk"]
    M = cfg["M"]
    CAP = M * 128
    nblk_g = cores * nblk
    np_big = np.float16 if cfg["host_f16"] else np.float32

    ds, ss, slot = layout["ds"], layout["ss"], layout["slot"]
    gb = layout["gb"]
    edge_feat = np.asarray(inputs["edge_feat"])
    feat = np.asarray(inputs["feat"])

    total = nblk_g * CAP
    dstloc = np.full(total, blk, dtype=np.int64)
    dstloc[slot] = ds - gb * blk

    ef_pad = np.zeros((total, D), dtype=np_big)
    ef_pad[slot] = edge_feat[layout["order"]].astype(np_big)
    fs_pad = np.zeros((total, D), dtype=np_big)
    fs_pad[slot] = feat[ss].astype(np_big)

    f8 = mybir.dt.np(F8)
    sb_ = np.arange(total) % CAP
    gb_s = np.arange(total) // CAP

    # transposed one-hot (k-expansion lhsT): ohT[b, n, j, p] = 1 iff
    # dst_local(edge at slot j*128+p of block b) == n
    ohT = np.zeros(nblk_g * 128 * CAP, dtype=f8)
    oh_idx = ((gb_s * 128 + dstloc) * (CAP // 128) + sb_ // 128) * 128 + sb_ % 128
    ohT[oh_idx] = 1.0
    ohT = ohT.reshape(nblk_g, 128, CAP // 128, 128)

    # edge-major one-hot (agg lhsT): ohE[b, p, j, n] = 1 iff dst_local == n
    ohE = np.zeros(nblk_g * CAP * 128, dtype=f8)
    ohE_idx = ((gb_s * CAP + sb_) * 128) + dstloc
    ohE[ohE_idx] = 1.0
    ohE = ohE.reshape(nblk_g, CAP // 128, 128, 128).transpose(0, 2, 1, 3)
    ohE = np.ascontiguousarray(ohE)

    per_core = nblk * CAP
    in_maps = []
    for c_i in range(cores):
        bsl = slice(c_i * nblk, (c_i + 1) * nblk)
        sl = slice(c_i * per_core, (c_i + 1) * per_core)
        m = {
            "eft": np.ascontiguousarray(ef_pad[sl].T),
            "fslotT": np.ascontiguousarray(fs_pad[sl].T),
            "ohT": np.ascontiguousarray(ohT[bsl]),
            "ohE": np.ascontiguousarray(ohE[bsl]),
            "featT": np.ascontiguousarray(
                feat[c_i * npc : (c_i + 1) * npc].astype(np_big).T
            ),
            "WoT": np.ascontiguousarray(np.asarray(inputs["Wo"]).T),
            "WskipT": np.ascontiguousarray(np.asarray(inputs["Wskip"]).T),
        }
        for name in ("Wq", "Wk", "Wv", "Wo", "Wskip", "W1", "W2", "We", "Wg",
                     "Wgate", "ln1_g", "ln1_b", "ln2_g", "ln2_b"):
            m[name] = np.ascontiguousarray(np.asarray(inputs[name]))
        in_maps.append(m)
    return in_maps


_cache = {}


def _get_program(cfg):
    key = (cfg["cores"], cfg["M"], cfg["host_f16"])
    if key not in _cache:
        _cache[key] = build_program(cfg)
    return _cache[key]


def full_base():
    return dict(cores=CORES, n_nodes=N_NODES, npc=NPC, nblk=NBLK, blk=BLK)


def _ensure_ntff_hook():
    """The agent image's antenv lacks axon_hooks; synthesize it from the
    boot module's ctypes NTFF profiler so trace=True can capture timings."""
    import types

    if "antenv.axon_hooks" in sys.modules:
        return
    try:
        sys.path.insert(0, "/root/.axon_site")
        from trn_agent_boot.trn_boot import _ntff_profile_via_ctypes

        hook = _ntff_profile_via_ctypes("/opt/axon/libaxon_pjrt.so")
        mod = types.ModuleType("antenv.axon_hooks")
        mod.get_axon_ntff_profile_hook = lambda: hook
        mod.set_axon_ntff_profile_hook = lambda h: None
        sys.modules["antenv.axon_hooks"] = mod
    except Exception as e:  # degrade to untimed run
        print(f"ntff hook setup failed: {e}")


def run(inputs, trace=False, tmpdir=None, trace_cores=None):
    if trace:
        _ensure_ntff_hook()
    cfg, layout = compute_layout(inputs, full_base())
    nc = _get_program(cfg)
    in_maps = shard_inputs(inputs, cfg, layout)
    res = bass_utils.run_bass_kernel_spmd(
        nc,
        in_maps,
        core_ids=list(range(cfg["cores"])),
        trace=trace,
        tmpdir=tmpdir,
        trace_cores=trace_cores,
    )
    out = np.concatenate([res.results[c]["out"] for c in range(cfg["cores"])], axis=0).astype(np.float32)
    return out, res


def kernel(**inputs):
    out, _ = run(inputs)
    return out


# revision 32
# speedup vs baseline: 1.0729x; 1.0524x over previous
"""Trainium2 Bass kernel for GAT-style GNN message passing (edge softmax).

Contract: kernel(**inputs) takes FULL unsharded numpy inputs, distributes
across 8 NeuronCores internally, returns FULL output.

Sharding: edges sorted by dst and partitioned by dst range (6250 nodes per
core) -> every per-destination segment reduction is core-local. Node
features/weights replicated.

v2 design notes (vs v1 baseline):
- All matmuls use f16/bf16/f8 operands (fp32 matmul is 4 cyc/row on PE).
- edge_feat projections (e_bias/gates) computed in the main block loop
  (no separate stage + DRAM round trip).
- f32->f16 conversion happens inside the DMA (SWDGE cast) - no on-chip
  cast instructions.
- q and v are consumed directly from PSUM by the DVE ops that need them
  (the op IS the drain); k is drained to f16 SBUF split ACT/gpsimd.
- Edge-major one-hot (agg lhsT) shipped from host as f8 instead of an
  on-device is_equal build.
- k for own nodes kept resident in SBUF (no DRAM round trip).
- LayerNorm restructured: Square(bias=-mu, accum) + folded affine.
"""

import os
import sys

sys.path.insert(0, "/opt/trn_rl_repo")

import numpy as np

import concourse.bass as bass
import concourse.mybir as mybir
import concourse.tile as tile
from concourse import bacc
from concourse import bass_utils
from concourse.masks import make_identity

F32 = mybir.dt.float32
F16 = mybir.dt.float16
BF16 = mybir.dt.bfloat16
F8 = mybir.dt.float8e4
I32 = mybir.dt.int32
AF = mybir.ActivationFunctionType
OP = mybir.AluOpType

D = 128
H = 8
HD = 16
EPS = 1e-5

N_NODES = 50000
N_EDGES = 800000
CORES = 8
NPC = N_NODES // CORES      # nodes per core = 6250
BLK = 125                   # dst nodes per block
NBLK = NPC // BLK           # 50 blocks per core

# If "1": host stages the big per-edge tables (edge_feat, feat[src]) in
# fp16 (same values the device math would see after its own cast).
HOST_F16 = os.environ.get("KERNEL_HOST_F16", "1") == "1"


def _force_act_set():
    """Pin every ACTIVATE to the natural_log_exp_and_others table so the
    kernel pays one ACT_TABLE_LOAD instead of hundreds."""
    from concourse import hw_specs

    if getattr(bacc, "_act_set_forced", False):
        return
    real = hw_specs.get_activation_tables

    def patched(arch):
        t = dict(real(arch))
        keep = "natural_log_exp_and_others"
        return {name: (fns if name == keep else set()) for name, fns in t.items()}

    bacc.get_activation_tables = patched
    bacc._act_set_forced = True


def build_program(cfg):
    _force_act_set()
    cores = cfg["cores"]
    npc = cfg["npc"]
    nblk = cfg["nblk"]
    blk = cfg["blk"]
    M = cfg["M"]
    CAP = M * 128
    EPC = nblk * CAP
    host_f16 = cfg["host_f16"]
    big_dt = F16 if host_f16 else F32

    nc = bacc.Bacc(
        "TRN2", target_bir_lowering=False, debug=False, num_devices=cores
    )

    # ---- I/O ----
    eft_d = nc.dram_tensor("eft", [D, EPC], big_dt, kind="ExternalInput").ap()
    fslotT_d = nc.dram_tensor("fslotT", [D, EPC], big_dt, kind="ExternalInput").ap()
    ohT_d = nc.dram_tensor("ohT", [nblk, 128, M, 128], F8, kind="ExternalInput").ap()
    ohE_d = nc.dram_tensor("ohE", [nblk, 128, M, 128], F8, kind="ExternalInput").ap()
    featT_d = nc.dram_tensor("featT", [D, npc], big_dt, kind="ExternalInput").ap()
    w_in = {}
    for name in ("Wq", "Wk", "Wv", "Wo", "Wskip", "W1", "W2", "WoT", "WskipT"):
        w_in[name] = nc.dram_tensor(name, [D, D], F32, kind="ExternalInput").ap()
    w_in["We"] = nc.dram_tensor("We", [D, H], F32, kind="ExternalInput").ap()
    w_in["Wg"] = nc.dram_tensor("Wg", [D, H], F32, kind="ExternalInput").ap()
    w_in["Wgate"] = nc.dram_tensor("Wgate", [3 * D, 1], F32, kind="ExternalInput").ap()
    for name in ("ln1_g", "ln1_b", "ln2_g", "ln2_b"):
        w_in[name] = nc.dram_tensor(name, [D], F32, kind="ExternalInput").ap()
    out_d = nc.dram_tensor("out", [npc, D], F32, kind="ExternalOutput").ap()

    with tile.TileContext(nc) as tc:
        import contextlib

        ctx = contextlib.ExitStack()
        with ctx:
            consts = ctx.enter_context(tc.tile_pool(name="consts", bufs=1))

            # ---------- setup ----------
            ident16 = consts.tile([128, 128], F16)
            make_identity(nc, ident16[:])

            ones_row = consts.tile([1, 128], F32)
            nc.vector.memset(ones_row[:], 1.0)

            zrow = consts.tile([128, D], F32)
            nc.vector.memset(zrow[:], 0.0)

            const2 = consts.tile([128, 2], F32)
            nc.vector.memset(const2[:, 0:1], 0.0)
            nc.vector.memset(const2[:, 1:2], EPS)
            nc.const_aps.aps[(F32, 0.0)] = const2[:, 0:1]
            nc.const_aps.aps[(F32, EPS)] = const2[:, 1:2]

            lnrow = consts.tile([1, 4 * D], F32)
            for i, name in enumerate(("ln1_g", "ln1_b", "ln2_g", "ln2_b")):
                nc.sync.dma_start(
                    out=lnrow[:, i * D : (i + 1) * D], in_=w_in[name][None, :]
                )
            lnb = consts.tile([128, 4 * D], F32)

            wqv32 = consts.tile([D, 2 * D], F32)
            nc.sync.dma_start(out=wqv32[:, 0:D], in_=w_in["Wq"][:])
            nc.sync.dma_start(out=wqv32[:, D : 2 * D], in_=w_in["Wv"][:])
            wqv16 = consts.tile([D, 2 * D], F16)
            nc.vector.tensor_copy(out=wqv16[:], in_=wqv32[:])
            wk16 = consts.tile([D, D], F16)
            nc.gpsimd.dma_start(out=wk16[:], in_=w_in["Wk"][:])
            weg32 = consts.tile([D, 2 * H], F32)
            nc.sync.dma_start(out=weg32[:, 0:H], in_=w_in["We"][:])
            nc.sync.dma_start(out=weg32[:, H : 2 * H], in_=w_in["Wg"][:])
            weg16 = consts.tile([D, 2 * H], F16)
            nc.vector.tensor_copy(out=weg16[:], in_=weg32[:])
            w1_16 = consts.tile([D, D], F16)
            nc.gpsimd.dma_start(out=w1_16[:], in_=w_in["W1"][:])
            w2_16 = consts.tile([D, D], F16)
            nc.gpsimd.dma_start(out=w2_16[:], in_=w_in["W2"][:])

            # gate vector folding: gate_pre = agg@(Wo@A) + feat@(Wskip@B)
            wg3 = consts.tile([128, 3], F32)
            nc.sync.dma_start(
                out=wg3[:], in_=w_in["Wgate"].rearrange("(t p) c -> p (t c)", p=128)
            )
            ab = consts.tile([128, 2], F32)
            nc.vector.tensor_add(out=ab[:, 0:1], in0=wg3[:, 0:1], in1=wg3[:, 2:3])
            nc.vector.tensor_sub(out=ab[:, 1:2], in0=wg3[:, 1:2], in1=wg3[:, 2:3])

            wot_s = consts.tile([D, D], F32)
            nc.sync.dma_start(out=wot_s[:], in_=w_in["WoT"][:])
            wskipt_s = consts.tile([D, D], F32)
            nc.sync.dma_start(out=wskipt_s[:], in_=w_in["WskipT"][:])

            rhs_o32 = consts.tile([D, D + 1], F32)
            nc.sync.dma_start(out=rhs_o32[:, 0:D], in_=w_in["Wo"][:])
            rhs_s32 = consts.tile([D, D + 1], F32)
            nc.sync.dma_start(out=rhs_s32[:, 1 : D + 1], in_=w_in["Wskip"][:])

            with tc.tile_pool(name="psum_setup", bufs=1, space="PSUM") as pss:
                ps_ln = pss.tile([128, 4 * D], F32, tag="ln")
                nc.tensor.matmul(
                    out=ps_ln[:], lhsT=ones_row[:], rhs=lnrow[:], start=True, stop=True
                )
                nc.vector.tensor_copy(out=lnb[:], in_=ps_ln[:])

                ps_c = pss.tile([128, 2], F32, tag="c")
                nc.tensor.matmul(
                    out=ps_c[:, 0:1], lhsT=wot_s[:], rhs=ab[:, 0:1],
                    start=True, stop=True,
                )
                nc.tensor.matmul(
                    out=ps_c[:, 1:2], lhsT=wskipt_s[:], rhs=ab[:, 1:2],
                    start=True, stop=True,
                )
                nc.vector.tensor_copy(out=rhs_o32[:, D : D + 1], in_=ps_c[:, 0:1])
                nc.vector.tensor_copy(out=rhs_s32[:, 0:1], in_=ps_c[:, 1:2])

            rhs_o16 = consts.tile([D, D + 1], F16)
            nc.vector.tensor_copy(out=rhs_o16[:], in_=rhs_o32[:])
            rhs_s16 = consts.tile([D, D + 1], F16)
            nc.vector.tensor_copy(out=rhs_s16[:], in_=rhs_s32[:])

            featT16 = consts.tile([D, npc], F16)
            if host_f16:
                nc.sync.dma_start(out=featT16[:], in_=featT_d[:])
            else:
                nc.gpsimd.dma_start(out=featT16[:], in_=featT_d[:])

            # ---------- stage 1: k for own nodes, SBUF-resident ----------
            # k_all[n_local, b, :] = (feat @ Wk)[b*blk + n_local, :]
            k_all = consts.tile([blk, nblk, D], F16)
            with tc.tile_pool(name="k1_ps", bufs=2, space="PSUM") as kps:
                for g0 in range(0, nblk, 4):
                    ng = min(4, nblk - g0)
                    ps_k1 = kps.tile([blk, 4, D], F32, tag="k1")
                    for j in range(ng):
                        b = g0 + j
                        nc.tensor.matmul(
                            out=ps_k1[:, j, :],
                            lhsT=featT16[:, b * blk : (b + 1) * blk],
                            rhs=wk16[:],
                            start=True, stop=True,
                        )
                    nc.scalar.activation(
                        out=k_all[:, g0 : g0 + ng, :], in_=ps_k1[:, 0:ng, :],
                        func=AF.Copy,
                    )

            # ---------- main loop ----------
            sb2 = ctx.enter_context(tc.tile_pool(name="p2_sb", bufs=3))
            sb2a = ctx.enter_context(tc.tile_pool(name="p2_sba", bufs=2))
            epi = ctx.enter_context(tc.tile_pool(name="epi_sb", bufs=2))
            # PSUM bank packing (8 banks x 2KB): q/v share one rotating tag
            # (2 banks), k 2 banks, eg+agg packed in one tile (2 banks),
            # tr 1 bank, rs 1 bank.
            ps_qvp = ctx.enter_context(tc.tile_pool(name="ps_qv", bufs=2, space="PSUM"))
            ps_kp = ctx.enter_context(tc.tile_pool(name="ps_k", bufs=2, space="PSUM"))
            ps_egp = ctx.enter_context(tc.tile_pool(name="ps_eg", bufs=2, space="PSUM"))
            ps_trp = ctx.enter_context(tc.tile_pool(name="ps_tr", bufs=2, space="PSUM"))

            def layer_norm(x_t, g_col, b_col, out_t, out_dt_tag, nb):
                """out = (x - mu) * rstd * g + b, via Square(bias=-mu, accum)."""
                nm = epi.tile([blk, 2], F32, tag=f"ln_nm{out_dt_tag}")
                nc.vector.tensor_reduce(
                    out=nm[:nb, 0:1], in_=x_t[:nb], axis=mybir.AxisListType.X,
                    op=OP.add, negate=True,
                )
                nc.vector.tensor_scalar_mul(
                    out=nm[:nb, 1:2], in0=nm[:nb, 0:1], scalar1=1.0 / D
                )
                sq = epi.tile([blk, D], F32, tag=f"ln_sq{out_dt_tag}")
                v2 = epi.tile([blk, 3], F32, tag=f"ln_v2{out_dt_tag}")
                nc.scalar.activation(
                    out=sq[:nb], in_=x_t[:nb], func=AF.Square,
                    bias=nm[:nb, 1:2], accum_out=v2[:nb, 0:1],
                )
                nc.scalar.activation(
                    out=v2[:nb, 1:2], in_=v2[:nb, 0:1], func=AF.Ln,
                    scale=1.0 / D, bias=EPS,
                )
                nc.scalar.activation(
                    out=v2[:nb, 2:3], in_=v2[:nb, 1:2], func=AF.Exp, scale=-0.5
                )
                t1 = epi.tile([blk, D], F32, tag=f"ln_t1{out_dt_tag}")
                nc.vector.scalar_tensor_tensor(
                    out=t1[:nb], in0=lnb[:nb, g_col * D : (g_col + 1) * D],
                    scalar=v2[:nb, 2:3], in1=zrow[:nb],
                    op0=OP.mult, op1=OP.add,
                )
                nc.vector.scalar_tensor_tensor(
                    out=out_t[:nb], in0=x_t[:nb], scalar=nm[:nb, 1:2],
                    in1=t1[:nb], op0=OP.add, op1=OP.mult,
                )
                nc.vector.tensor_add(
                    out=out_t[:nb], in0=out_t[:nb],
                    in1=lnb[:nb, b_col * D : (b_col + 1) * D],
                )

            for b0 in range(0, nblk, 2):
                # pair-wide tiles: the softmax mid-chain runs once per 2 blocks
                qk16 = sb2a.tile([128, 2, M, 128], F16, tag="qk")
                pu = sb2a.tile([128, 2, M, H + D], BF16, tag="pu")
                eg16 = sb2.tile([128, 2, M, 2 * H], F16, tag="eg16")
                fronts = []
                for bi in range(2):
                    b = b0 + bi
                    fs16 = sb2.tile([128, CAP], F16, tag="fs16")
                    ef16 = sb2.tile([128, CAP], F16, tag="ef16")
                    if host_f16:
                        nc.sync.dma_start(
                            out=fs16[:], in_=fslotT_d[:, b * CAP : (b + 1) * CAP]
                        )
                        nc.sync.dma_start(
                            out=ef16[:], in_=eft_d[:, b * CAP : (b + 1) * CAP]
                        )
                    else:
                        nc.gpsimd.dma_start(
                            out=fs16[:], in_=fslotT_d[:, b * CAP : (b + 1) * CAP]
                        )
                        nc.gpsimd.dma_start(
                            out=ef16[:], in_=eft_d[:, b * CAP : (b + 1) * CAP]
                        )
                    ohT_t = sb2.tile([128, M, 128], F8, tag="oht")
                    nc.sync.dma_start(out=ohT_t[:], in_=ohT_d[b])
                    ohE_t = sb2.tile([128, M, 128], F8, tag="ohe")
                    nc.sync.dma_start(out=ohE_t[:], in_=ohE_d[b])

                    # eg = edge_feat @ [We|Wg], whole block in one PSUM bank
                    egagg = ps_egp.tile([128, 512], F32, tag="egagg")
                    ps_eg = egagg[:, 0 : M * 2 * H].rearrange(
                        "p (m c) -> p m c", c=2 * H
                    )
                    for j in range(M):
                        nc.tensor.matmul(
                            out=ps_eg[:, j, :],
                            lhsT=ef16[:, j * 128 : (j + 1) * 128],
                            rhs=weg16[:],
                            start=True, stop=True,
                        )
                    nc.scalar.activation(out=eg16[:, bi], in_=ps_eg, func=AF.Copy)

                    # k expansion on PE, drained f16 (ACT/DVE split)
                    k16 = sb2.tile([128, M, 128], F16, tag="k16")
                    kgrps = []
                    for gi, g0 in enumerate(range(0, M, 4)):
                        ng = min(4, M - g0)
                        ps_k = ps_kp.tile([128, 4, 128], F32, tag="k")
                        for jj in range(ng):
                            nc.tensor.matmul(
                                out=ps_k[:, jj, :],
                                lhsT=ohT_t[:blk, g0 + jj, :],
                                rhs=k_all[:, b, :],
                                start=True, stop=True,
                            )
                        kgrps.append((gi, g0, ng, ps_k))
                    for gi, g0, ng, ps_k in kgrps:
                        if gi % 2 == 0:
                            nc.scalar.activation(
                                out=k16[:, g0 : g0 + ng, :], in_=ps_k[:, 0:ng, :],
                                func=AF.Copy,
                            )
                        else:
                            nc.vector.tensor_copy(
                                out=k16[:, g0 : g0 + ng, :], in_=ps_k[:, 0:ng, :]
                            )

                    # q per edge (PSUM-resident, consumed by qk mul)
                    for g0 in range(0, M, 4):
                        ng = min(4, M - g0)
                        ps_q = ps_qvp.tile([128, 4, 128], F32, tag="qv")
                        for jj in range(ng):
                            nc.tensor.matmul(
                                out=ps_q[:, jj, :],
                                lhsT=fs16[:, (g0 + jj) * 128 : (g0 + jj + 1) * 128],
                                rhs=wqv16[:, 0:D],
                                start=True, stop=True,
                            )
                        nc.vector.tensor_mul(
                            out=qk16[:, bi, g0 : g0 + ng, :],
                            in0=ps_q[:, 0:ng, :],
                            in1=k16[:, g0 : g0 + ng, :],
                        )
                    fronts.append((b, fs16, ohE_t, egagg))

                # --- pair mid-chain: per-head dot -> logits -> p, gates ---
                qk8 = sb2a.tile([128, 2 * M * H, HD // 2], F16, tag="qk8")
                qkv = qk16[:].rearrange(
                    "p b m (h f x) -> p (b m h) f x", f=2, x=HD // 2
                )
                nc.vector.tensor_add(
                    out=qk8[:], in0=qkv[:, :, 0, :], in1=qkv[:, :, 1, :]
                )
                a32 = sb2a.tile([128, 2 * M * H], F32, tag="a")
                nc.vector.tensor_reduce(
                    out=a32[:],
                    in_=qk8[:],
                    axis=mybir.AxisListType.X,
                    op=OP.add,
                )
                w16 = sb2a.tile([128, 2, M, H], F16, tag="w")
                nc.vector.tensor_scalar(
                    out=w16[:].rearrange("p b m h -> p (b m h)"), in0=a32[:],
                    scalar1=5.0, scalar2=-5.0, op0=OP.min, op1=OP.max,
                )
                nc.vector.tensor_add(
                    out=w16[:], in0=w16[:], in1=eg16[:, :, :, 0:H]
                )
                nc.scalar.activation(
                    out=pu[:, :, :, 0:H], in_=w16[:], func=AF.Exp, scale=4.0
                )
                sg16 = sb2a.tile([128, 2, M, H], BF16, tag="sg")
                nc.scalar.activation(
                    out=sg16[:], in_=eg16[:, :, :, H : 2 * H], func=AF.Exp,
                    scale=-1.0,
                )
                nc.vector.tensor_scalar_add(out=sg16[:], in0=sg16[:], scalar1=1.0)
                gate16 = sb2a.tile([128, 2, M, H], BF16, tag="gate")
                with nc.allow_low_precision(reason="sigmoid gate, |err|<0.4%"):
                    nc.vector.reciprocal(out=gate16[:], in_=sg16[:])
                pg = sb2a.tile([128, 2, M, H], BF16, tag="pg")
                nc.vector.tensor_mul(out=pg[:], in0=pu[:, :, :, 0:H], in1=gate16[:])

                assert M * 2 * H + (H + D) <= 512
                for bi in range(2):
                    b, fs16, ohE_t, egagg = fronts[bi]
                    ps_agg = egagg[:, M * 2 * H : M * 2 * H + H + D]
                    # v per edge + weighted aggregate
                    vgrps = []
                    for g0 in range(0, M, 4):
                        ng = min(4, M - g0)
                        ps_v = ps_qvp.tile([128, 4, 128], F32, tag="qv")
                        for jj in range(ng):
                            nc.tensor.matmul(
                                out=ps_v[:, jj, :],
                                lhsT=fs16[:, (g0 + jj) * 128 : (g0 + jj + 1) * 128],
                                rhs=wqv16[:, D : 2 * D],
                                start=True, stop=True,
                            )
                        vgrps.append((g0, ng, ps_v))
                    for g0, ng, ps_v in vgrps:
                        nc.vector.tensor_mul(
                            out=pu[:, bi, g0 : g0 + ng, H : H + D].rearrange(
                                "p m (h x) -> p m h x", x=HD
                            ),
                            in0=ps_v[:, 0:ng, :].rearrange(
                                "p m (h x) -> p m h x", x=HD
                            ),
                            in1=pg[:, bi, g0 : g0 + ng, :, None].to_broadcast(
                                [128, ng, H, HD]
                            ),
                        )
                        for jj in range(ng):
                            j = g0 + jj
                            nc.tensor.matmul(
                                out=ps_agg,
                                lhsT=ohE_t[:, j, :],
                                rhs=pu[:, bi, j, :],
                                start=(j == 0),
                                stop=(j == M - 1),
                            )

                    # node epilogue
                    nb = blk
                    dinv = epi.tile([blk, 2 * H], F32, tag="dinv")
                    nc.vector.tensor_scalar_max(
                        out=dinv[:nb, 0:H], in0=ps_agg[:nb, 0:H], scalar1=1e-30
                    )
                    nc.vector.reciprocal(out=dinv[:nb, H : 2 * H], in_=dinv[:nb, 0:H])
                    agg16 = epi.tile([blk, D], F16, tag="agg16")
                    nc.vector.tensor_mul(
                        out=agg16[:nb].rearrange("p (h x) -> p h x", x=HD),
                        in0=ps_agg[:nb, H : H + D].rearrange("p (h x) -> p h x", x=HD),
                        in1=dinv[:nb, H : 2 * H, None].to_broadcast([nb, H, HD]),
                    )

                    trrs = ps_trp.tile([128, 512], F32, tag="trrs")
                    ps_tr = trrs[:, 264:456].bitcast(F16).rearrange(
                        "p (s c) -> p s c", c=128
                    )
                    nc.tensor.transpose(
                        out=ps_tr[:, 0, 0:blk], in_=agg16[:nb],
                        identity=ident16[:nb, :nb],
                    )
                    aggT16 = epi.tile([D, blk], F16, tag="aggT")
                    nc.vector.tensor_copy(out=aggT16[:], in_=ps_tr[:, 0, 0:blk])

                    rsf = trrs[:blk, 0 : 2 * D + 2]
                    nc.tensor.matmul(
                        out=rsf[:nb, 0 : D + 1], lhsT=aggT16[:, :nb], rhs=rhs_o16[:],
                        start=True, stop=True,
                    )
                    nc.tensor.matmul(
                        out=rsf[:nb, D + 1 : 2 * D + 2],
                        lhsT=featT16[:, b * blk : b * blk + nb],
                        rhs=rhs_s16[:],
                        start=True, stop=True,
                    )
                    sk32 = epi.tile([blk, D + 1], F32, tag="sk")
                    nc.scalar.activation(
                        out=sk32[:nb], in_=rsf[:nb, D + 1 : 2 * D + 2], func=AF.Copy
                    )
                    gprc = epi.tile([blk, 3], F32, tag="gprc")
                    nc.vector.tensor_add(
                        out=gprc[:nb, 0:1], in0=rsf[:nb, D : D + 1], in1=sk32[:nb, 0:1]
                    )
                    nc.scalar.activation(
                        out=gprc[:nb, 1:2], in_=gprc[:nb, 0:1], func=AF.Exp, scale=-1.0
                    )
                    nc.vector.tensor_scalar_add(
                        out=gprc[:nb, 1:2], in0=gprc[:nb, 1:2], scalar1=1.0
                    )
                    nc.vector.reciprocal(out=gprc[:nb, 2:3], in_=gprc[:nb, 1:2])
                    diff = epi.tile([blk, D], F32, tag="diff")
                    nc.vector.tensor_sub(
                        out=diff[:nb], in0=rsf[:nb, 0:D], in1=sk32[:nb, 1 : D + 1]
                    )
                    mix = epi.tile([blk, D], F32, tag="mix")
                    nc.vector.scalar_tensor_tensor(
                        out=mix[:nb], in0=diff[:nb], scalar=gprc[:nb, 2:3],
                        in1=sk32[:nb, 1 : D + 1],
                        op0=OP.mult, op1=OP.add,
                    )

                    h32 = epi.tile([blk, D], F32, tag="h")
                    layer_norm(mix, 0, 1, h32, "1", nb)
                    l216 = epi.tile([blk, D], F16, tag="l2")
                    layer_norm(h32, 2, 3, l216, "2", nb)

                    nc.tensor.transpose(
                        out=ps_tr[:, 1, 0:blk], in_=l216[:nb],
                        identity=ident16[:nb, :nb],
                    )
                    l2T16 = epi.tile([D, blk], F16, tag="l2T")
                    nc.vector.tensor_copy(out=l2T16[:], in_=ps_tr[:, 1, 0:blk])
                    nc.tensor.matmul(
                        out=rsf[:nb, D + 2 : 2 * D + 2], lhsT=l2T16[:, :nb],
                        rhs=w1_16[:],
                        start=True, stop=True,
                    )
                    r16 = epi.tile([blk, D], F16, tag="r")
                    nc.scalar.activation(
                        out=r16[:nb], in_=rsf[:nb, D + 2 : 2 * D + 2], func=AF.Relu
                    )
                    nc.tensor.transpose(
                        out=ps_tr[:, 2, 0:blk], in_=r16[:nb],
                        identity=ident16[:nb, :nb],
                    )
                    rT16 = epi.tile([D, blk], F16, tag="rT")
                    nc.vector.tensor_copy(out=rT16[:], in_=ps_tr[:, 2, 0:blk])
                    nc.tensor.matmul(
                        out=rsf[:nb, 0:D], lhsT=rT16[:, :nb], rhs=w2_16[:],
                        start=True, stop=True,
                    )
                    outb = epi.tile([blk, D], F32, tag="outb")
                    nc.vector.tensor_add(
                        out=outb[:nb], in0=h32[:nb], in1=rsf[:nb, 0:D]
                    )
                    nc.scalar.dma_start(
                        out=out_d[b * blk : b * blk + nb, :], in_=outb[:nb]
                    )

    nc.compile()
    return nc


def compute_layout(inputs, base):
    """Decide the data-dependent static block capacity M (tiles per block)."""
    cores, npc, nblk, blk = base["cores"], base["npc"], base["nblk"], base["blk"]
    nblk_g = cores * nblk

    src = np.asarray(inputs["src"]).astype(np.int64)
    dst = np.asarray(inputs["dst"]).astype(np.int64)
    gb_all = dst // blk
    order = np.lexsort((src, gb_all))  # by block, then src
    ds = dst[order]
    ss = src[order]
    gb = gb_all[order]

    counts = np.bincount(gb, minlength=nblk_g)
    M = max(2, int(np.ceil(counts.max() / 128)))

    starts = np.zeros(nblk_g + 1, dtype=np.int64)
    np.cumsum(counts, out=starts[1:])
    pos = np.arange(len(ds)) - starts[gb]
    slot = gb * (M * 128) + pos

    layout = dict(order=order, ds=ds, ss=ss, gb=gb, slot=slot)
    cfg = dict(base, M=M, host_f16=HOST_F16)
    return cfg, layout


def shard_inputs(inputs, cfg, layout):
    """Host-side layout only (sort/pad/transpose/index; dtype staging)."""
    cores = cfg["cores"]
    npc = cfg["npc"]
    nblk = cfg["nblk"]
    blk = cfg["blk"]
    M = cfg["M"]
    CAP = M * 128
    nblk_g = cores * nblk
    np_big = np.float16 if cfg["host_f16"] else np.float32

    ds, ss, slot = layout["ds"], layout["ss"], layout["slot"]
    gb = layout["gb"]
    edge_feat = np.asarray(inputs["edge_feat"])
    feat = np.asarray(inputs["feat"])

    total = nblk_g * CAP
    dstloc = np.full(total, blk, dtype=np.int64)
    dstloc[slot] = ds - gb * blk

    ef_pad = np.zeros((total, D), dtype=np_big)
    ef_pad[slot] = edge_feat[layout["order"]].astype(np_big)
    fs_pad = np.zeros((total, D), dtype=np_big)
    fs_pad[slot] = feat[ss].astype(np_big)

    f8 = mybir.dt.np(F8)
    sb_ = np.arange(total) % CAP
    gb_s = np.arange(total) // CAP

    # transposed one-hot (k-expansion lhsT): ohT[b, n, j, p] = 1 iff
    # dst_local(edge at slot j*128+p of block b) == n
    ohT = np.zeros(nblk_g * 128 * CAP, dtype=f8)
    oh_idx = ((gb_s * 128 + dstloc) * (CAP // 128) + sb_ // 128) * 128 + sb_ % 128
    ohT[oh_idx] = 1.0
    ohT = ohT.reshape(nblk_g, 128, CAP // 128, 128)

    # edge-major one-hot (agg lhsT): ohE[b, p, j, n] = 1 iff dst_local == n
    ohE = np.zeros(nblk_g * CAP * 128, dtype=f8)
    ohE_idx = ((gb_s * CAP + sb_) * 128) + dstloc
    ohE[ohE_idx] = 1.0
    ohE = ohE.reshape(nblk_g, CAP // 128, 128, 128).transpose(0, 2, 1, 3)
    ohE = np.ascontiguousarray(ohE)

    per_core = nblk * CAP
    in_maps = []
    for c_i in range(cores):
        bsl = slice(c_i * nblk, (c_i + 1) * nblk)
        sl = slice(c_i * per_core, (c_i + 1) * per_core)
        m = {
            "eft": np.ascontiguousarray(ef_pad[sl].T),
            "fslotT": np.ascontiguousarray(fs_pad[sl].T),
            "ohT": np.ascontiguousarray(ohT[bsl]),
            "ohE": np.ascontiguousarray(ohE[bsl]),
            "featT": np.ascontiguousarray(
                feat[c_i * npc : (c_i + 1) * npc].astype(np_big).T
            ),
            "WoT": np.ascontiguousarray(np.asarray(inputs["Wo"]).T),
            "WskipT": np.ascontiguousarray(np.asarray(inputs["Wskip"]).T),
        }
        for name in ("Wq", "Wk", "Wv", "Wo", "Wskip", "W1", "W2", "We", "Wg",
                     "Wgate", "ln1_g", "ln1_b", "ln2_g", "ln2_b"):
            m[name] = np.ascontiguousarray(np.asarray(inputs[name]))
        in_maps.append(m)
    return in_maps


_cache = {}


def _get_program(cfg):
    key = (cfg["cores"], cfg["M"], cfg["host_f16"])
    if key not in _cache:
        _cache[key] = build_program(cfg)
    return _cache[key]


def full_base():
    return dict(cores=CORES, n_nodes=N_NODES, npc=NPC, nblk=NBLK, blk=BLK)


def _ensure_ntff_hook():
    """The agent image's antenv lacks axon_hooks; synthesize it from the
    boot module's ctypes NTFF profiler so trace=True can capture timings."""
    import types

    if "antenv.axon_hooks" in sys.modules:
        return
    try:
        sys.path.insert(0, "/root/.axon_site")
        from trn_agent_boot.trn_boot import _ntff_profile_via_ctypes

        hook = _ntff_profile_via_ctypes("/opt/axon/libaxon_pjrt.so")
        mod = types.ModuleType("antenv.axon_hooks")
        mod.get_axon_ntff_profile_hook = lambda: hook
        mod.set_axon_ntff_profile_hook = lambda h: None
        sys.modules["antenv.axon_hooks"] = mod
    except Exception as e:  # degrade to untimed run
        print(f"ntff hook setup failed: {e}")


def run(inputs, trace=False, tmpdir=None, trace_cores=None):
    if trace:
        _ensure_ntff_hook()
    cfg, layout = compute_layout(inputs, full_base())
    nc = _get_program(cfg)
    in_maps = shard_inputs(inputs, cfg, layout)
    res = bass_utils.run_bass_kernel_spmd(
        nc,
        in_maps,
        core_ids=list(range(cfg["cores"])),
        trace=trace,
        tmpdir=tmpdir,
        trace_cores=trace_cores,
    )
    out = np.concatenate([res.results[c]["out"] for c in range(cfg["cores"])], axis=0)
    return out, res


def kernel(**inputs):
    out, _ = run(inputs)
    return out
